# revision 3
# baseline (speedup 1.0000x reference)
"""Realspace Ewald sum on 8 Trainium2 NeuronCores — v3 (PE+ACT pipeline).

pot = NORM/(4*pi) * sum_{i!=j} q_i q_j erf(d_ij/sqrt2)/d_ij   (N=6144)

Key identity: erf(d/sqrt2)/d = sqrt(2/pi) * int_0^1 exp(-d^2 u^2/2) du.
Gauss-Legendre M=6 on [0,1] turns the kernel into a sum of 6 exponentials
in s = d^2 (max rel err 2.4e-3 over s in [1e-4, 80] — tolerance is 2e-2).

Per (i-tile, k) the device computes
  acc[i] = sum_j |q_j| * exp(-a_k * s_ij)
entirely on PE + ACT:
 - PE matmul (K=8, f16 inputs, f32 PSUM): s_k(i,j) = s_ij + gamma_k ln|q_j|
   via augmented coordinates. Rows: [-2x,-2y,-2z, Rhi_i, Rlo_i, 1, 1,
   gamma_k] x [x_j, y_j, z_j, 1, 1, Rhi_j, Rlo_j, ln|q_j|].  The |r|^2
   compensated hi/lo pair makes s_ii ~ 1e-5 so the (large) diagonal term
   can be subtracted exactly on the host.
 - ACT: exp with scale=-a'_k and accum_out -> per-partition sums over j.
   -a'_k * gamma_k == 1 exactly (gamma stored f16, scale = -1/float(gamma)),
   so the ln|q_j| row contributes exactly |q_j| as a free-axis weight.
 - sign of q_j: atoms sorted negatives-first into column blocks
   [0, 3584) / [3584, 7168) with e^{-100} zero-weight padding; each ACT
   accumulator column is single-signed, host subtracts.
No DVE work, no partition-broadcast DMAs (the v2 bottlenecks): inputs are
two small [8, W] f16 tensors; output is one [128, 144] f32 accumulator.
"""

import numpy as np

import concourse.bass as bass
import concourse.bacc as bacc
import concourse.mybir as mybir
import concourse.tile as tile
from concourse.bass_utils import run_bass_kernel_spmd

# ---------------------------------------------------------------- constants
N = 6144
P = 128
NCORES = 8
NTILE = N // P                    # 48 i-tiles
TILES_PER_CORE = NTILE // NCORES  # 6

HALF = 3584                       # columns per sign block (28 banks-of-128)
NP = 2 * HALF                     # 7168 padded j-columns
NBLK = 4                          # ACT blocks per (tile, k)
BLKW = NP // NBLK                 # 1792 = 3.5 PSUM banks
MMW = 512                         # matmul width (1 PSUM bank of f32)
MQ = 6                            # quadrature terms
K = 8                             # matmul contraction rows

LN_PAD = -100.0                   # ln|q| for padding columns -> e^-100 = 0

TWOPI = 2.0 * np.pi
NORM_FACTOR = 90.0474

F32 = mybir.dt.float32
F16 = mybir.dt.float16

# Gauss-Legendre M=6 on [0,1]: f(s) ~= sum_k C_K[k] * exp(-A_K[k] * s)
_x, _w = np.polynomial.legendre.leggauss(MQ)
_u = 0.5 * (_x + 1.0)
A_K = (0.5 * _u * _u).astype(np.float64)          # exponents
C_K = (np.sqrt(2.0 / np.pi) * 0.5 * _w).astype(np.float64)  # weights
# gamma_k stored in f16; use scale s.t. scale*gamma == -(-1) exactly in f32
GAMMA_H = np.array([np.float16(-1.0 / a) for a in A_K])
A_EFF = np.array([-1.0 / np.float64(g) for g in GAMMA_H])   # actual exponents


# ------------------------------------------------------------- bass program
def _build_bass(rep=1):
    nc = bacc.Bacc("TRN2", target_bir_lowering=False, debug=False,
                   num_devices=NCORES)
    bmov_d = nc.declare_dram_parameter("bmov", [K, NP], F16, isOutput=False)
    stat_d = nc.declare_dram_parameter(
        "stat", [K, TILES_PER_CORE * MQ * P], F16, isOutput=False)
    acc_d = nc.declare_dram_parameter(
        "acc", [P, TILES_PER_CORE * MQ * NBLK], F32, isOutput=True)

    nmm_full = BLKW // MMW            # 3 full-width matmuls per block
    mm_tail = BLKW - nmm_full * MMW   # + one 256-wide

    with tile.TileContext(nc) as tc:
        with (
            tc.tile_pool(name="sb", bufs=1) as sb,
            tc.tile_pool(name="ps", bufs=2, space="PSUM") as ps,
        ):
            for r_ in range(rep):
                bmov = sb.tile([K, NP], F16, name="bmov", tag="bmov")
                stat = sb.tile([K, TILES_PER_CORE * MQ * P], F16,
                               name="stat", tag="stat")
                nc.sync.dma_start(out=bmov[:, :], in_=bmov_d.ap())
                nc.sync.dma_start(out=stat[:, :], in_=stat_d.ap())
                acc = sb.tile([P, TILES_PER_CORE * MQ * NBLK], F32,
                              name="acc", tag="acc")
                junk = sb.tile([P, BLKW], F16, name="junk", tag="junk")

                for t_ in range(TILES_PER_CORE):
                    for kq in range(MQ):
                        st0 = (t_ * MQ + kq) * P
                        lhsT = stat[:, st0:st0 + P]
                        scale = float(-A_EFF[kq])
                        for b in range(NBLK):
                            w0 = b * BLKW
                            s_ps = ps.tile([P, BLKW], F32,
                                           name=f"s{t_}{kq}{b}", tag="s")
                            for m in range(nmm_full):
                                o = m * MMW
                                nc.tensor.matmul(
                                    s_ps[:, o:o + MMW], lhsT,
                                    bmov[:, w0 + o:w0 + o + MMW],
                                    start=True, stop=True)
                            if mm_tail:
                                o = nmm_full * MMW
                                nc.tensor.matmul(
                                    s_ps[:, o:o + mm_tail], lhsT,
                                    bmov[:, w0 + o:w0 + o + mm_tail],
                                    start=True, stop=True)
                            col = (t_ * MQ + kq) * NBLK + b
                            nc.scalar.activation(
                                out=junk[:, :], in_=s_ps[:, :],
                                func=mybir.ActivationFunctionType.Exp,
                                bias=0.0, scale=scale,
                                accum_out=acc[:, col:col + 1])
                nc.sync.dma_start(out=acc_d.ap(), in_=acc[:, :])
    nc.compile()
    return nc


_CACHE = {}


def _get_nc(rep=1):
    key = ("nc", rep)
    if key not in _CACHE:
        _CACHE[key] = _build_bass(rep=rep)
    return _CACHE[key]


# ------------------------------------------------------------- host packing
def _pack_inputs(q, r):
    q = np.asarray(q, dtype=np.float32).reshape(-1)
    r = np.asarray(r, dtype=np.float32)

    neg = np.where(q < 0)[0]
    pos = np.where(q >= 0)[0]
    assert len(neg) <= HALF and len(pos) <= HALF, "sign blocks overflow"
    order = np.concatenate([neg, pos])

    qs = q[order]
    rs = r[order]
    xh = rs[:, 0].astype(np.float16)
    yh = rs[:, 1].astype(np.float16)
    zh = rs[:, 2].astype(np.float16)
    xf = xh.astype(np.float32)
    yf = yh.astype(np.float32)
    zf = zh.astype(np.float32)
    R32 = xf * xf + yf * yf + zf * zf
    Rhi = R32.astype(np.float16)
    Rlo = (R32 - Rhi.astype(np.float32)).astype(np.float16)
    with np.errstate(divide="ignore"):
        lnq = np.maximum(np.log(np.abs(qs)), LN_PAD).astype(np.float16)

    nneg = len(neg)
    nposv = len(pos)

    # row pairing (out_ij = sum_r stat[r,i] * B[r,j]):
    #   r0: -2x_i * x_j    r1: -2y_i * y_j    r2: -2z_i * z_j
    #   r3: Rhi_i * 1      r4: Rlo_i * 1
    #   r5: 1 * Rhi_j      r6: 1 * Rlo_j      r7: gamma_k * ln|q_j|
    # moving matrix: same for every core
    B = np.zeros((K, NP), np.float32)
    B[3, :] = 1.0
    B[4, :] = 1.0
    B[7, :] = LN_PAD
    slots = np.concatenate([np.arange(nneg),
                            HALF + np.arange(nposv)])
    B[0, slots] = xf
    B[1, slots] = yf
    B[2, slots] = zf
    B[5, slots] = Rhi.astype(np.float32)
    B[6, slots] = Rlo.astype(np.float32)
    B[7, slots] = lnq.astype(np.float32)
    Bh = B.astype(np.float16)

    in_maps = []
    for c in range(NCORES):
        stat = np.zeros((K, TILES_PER_CORE * MQ * P), np.float32)
        for t_ in range(TILES_PER_CORE):
            i0 = (c * TILES_PER_CORE + t_) * P
            sl = slice(i0, i0 + P)
            for kq in range(MQ):
                s0 = (t_ * MQ + kq) * P
                stat[0, s0:s0 + P] = -2.0 * xf[sl]
                stat[1, s0:s0 + P] = -2.0 * yf[sl]
                stat[2, s0:s0 + P] = -2.0 * zf[sl]
                stat[3, s0:s0 + P] = Rhi[sl].astype(np.float32)
                stat[4, s0:s0 + P] = Rlo[sl].astype(np.float32)
                stat[5, s0:s0 + P] = 1.0
                stat[6, s0:s0 + P] = 1.0
                stat[7, s0:s0 + P] = np.float32(GAMMA_H[kq])
        in_maps.append({"bmov": Bh, "stat": stat.astype(np.float16)})
    return in_maps, order, qs, lnq


# ------------------------------------------------------------------- kernel
def kernel(q, r, cell):
    in_maps, order, qs, lnq = _pack_inputs(q, r)
    nc = _get_nc(rep=1)
    res = run_bass_kernel_spmd(nc, in_maps, list(range(NCORES)))

    qs64 = qs.astype(np.float64)
    wq = np.exp(lnq.astype(np.float64))   # device j-weights, f16-rounded

    pot_raw = 0.0
    for c in range(NCORES):
        acc = res.results[c]["acc"].astype(np.float64)
        acc = acc.reshape(P, TILES_PER_CORE, MQ, NBLK)
        z = (acc[..., 2] + acc[..., 3]) - (acc[..., 0] + acc[..., 1])
        # y[p, t] = sum_k c_k z[p, t, k]
        y = np.einsum("ptk,k->pt", z, C_K)
        for t_ in range(TILES_PER_CORE):
            i0 = (c * TILES_PER_CORE + t_) * P
            pot_raw += float((qs64[i0:i0 + P] * y[:, t_]).sum())

    diag = float(C_K.sum()) * float((np.abs(qs64) * wq).sum())
    pot = (pot_raw - diag) / TWOPI / 2.0 * NORM_FACTOR
    return np.array([pot], dtype=np.float32)


def timed_run(inputs, iters=10, rep_hi=5):
    """Differential HW timing: interleaved rep=1 / rep=rep_hi wall samples;
    per-neighbor-pair differences; median + min-based estimates."""
    import time

    in_maps, _, _, _ = _pack_inputs(inputs["q"], inputs["r"])
    nc_lo = _get_nc(rep=1)
    nc_hi = _get_nc(rep=rep_hi)
    for nc in (nc_lo, nc_hi):
        for _ in range(2):
            run_bass_kernel_spmd(nc, in_maps, list(range(NCORES)))
    diffs = []
    lo_s, hi_s = [], []
    for it in range(iters):
        t0 = time.perf_counter()
        run_bass_kernel_spmd(nc_lo, in_maps, list(range(NCORES)))
        lo = time.perf_counter() - t0
        t0 = time.perf_counter()
        run_bass_kernel_spmd(nc_hi, in_maps, list(range(NCORES)))
        hi = time.perf_counter() - t0
        lo_s.append(lo)
        hi_s.append(hi)
        diffs.append((hi - lo) / (rep_hi - 1))
    diffs.sort()
    med = diffs[len(diffs) // 2]
    alt = (min(hi_s) - min(lo_s)) / (rep_hi - 1)
    ns = min(med, alt) if alt > 0 else med
    globals()["_LAST_WALLS"] = {1: min(lo_s), rep_hi: min(hi_s)}
    return int(ns * 1e9)
